# revision 24
# baseline (speedup 1.0000x reference)
"""Trainium2 Bass kernel for nn_Attention_b (tanh-attention with masked_scatter).

Data-parallel over batch: each of 8 NeuronCores owns 4 batches. Per core:
  phase 1  z = W1 @ h_i + (W2 @ h_t + b)   (fp32r GEMM, [A, rows])
           m = tanh(z); y = u . m          (raw scores, [rows])
  comm     AllGather of the per-chunk score slice across the 8 cores
  phase 2  masked_scatter selection (0/1 matrix against gathered scores)
           + online softmax over the sequence (flash-style, chunked)
  phase 3  s_acc += e * h_i  (fused DVE multiply-reduce on the resident
           h_i chunk -- h_i is read from HBM exactly once)
"""
import sys

for _p in ("/opt/trn_rl_repo",):
    if _p not in sys.path:
        sys.path.insert(0, _p)

import numpy as np

import concourse.bacc as bacc
import concourse.tile as tile
from concourse import mybir
from concourse.bass_utils import run_bass_kernel_spmd
from concourse.dve_ops import TENSOR_TENSOR_REDUCE
from concourse.masks import make_identity

NCORES = 8
B, S, H, A = 32, 2048, 1024, 256
BL = B // NCORES          # local batches per core
NEG = np.float32(-1e20)

f32 = mybir.dt.float32
f32r = mybir.dt.float32r


def build_kernel(S=S, H=H, A=A, C=256, hi_bufs=4):
    KT = H // 128             # contraction tiles
    AT = A // 128             # score tiles
    NCH = S // C              # sequence chunks
    assert S % C == 0 and H % 128 == 0 and A % 128 == 0 and C % 2 == 0

    nc = bacc.Bacc("TRN2", target_bir_lowering=False, debug=False,
                   num_devices=NCORES)

    # big operands are declared float32r (same bits as f32) so the plain
    # HWDGE DMA path can be used -- no SWDGE cast, no Q7 descriptor work
    hi5 = nc.declare_dram_parameter("hi5", [NCH, 128, KT * BL * C], f32r,
                                    isOutput=False)
    w1t = nc.declare_dram_parameter("w1t", [H, A], f32r, isOutput=False)
    cb2 = nc.declare_dram_parameter("cb2", [128, AT, BL], f32, isOutput=False)
    u2 = nc.declare_dram_parameter("u2", [128, AT], f32r, isOutput=False)
    sel = nc.declare_dram_parameter("sel", [B + 1, BL, S], f32,
                                    isOutput=False)
    out = nc.declare_dram_parameter("out", [BL, H], f32, isOutput=True)

    with tile.TileContext(nc) as tc:
        with (
            tc.tile_pool(name="consts", bufs=1) as cp,
            tc.tile_pool(name="hi", bufs=hi_bufs) as hip,
            tc.tile_pool(name="m", bufs=2) as mp,
            tc.tile_pool(name="small", bufs=3) as sp,
            tc.tile_pool(name="ebc", bufs=2) as ebp,
            tc.tile_pool(name="sacc", bufs=2) as sap,
            tc.tile_pool(name="pz", bufs=2, space="PSUM") as pz,
            tc.tile_pool(name="py", bufs=2, space="PSUM") as py,
            tc.tile_pool(name="dram", bufs=NCH, space="DRAM") as dp,
        ):
            # ---- preload replicated constants
            w1_sb = cp.tile([128, KT, A], f32r)
            nc.sync.dma_start(
                out=w1_sb, in_=w1t.rearrange("(t p) a -> p t a", p=128))
            u_sb = cp.tile([128, AT], f32r)
            nc.sync.dma_start(out=u_sb, in_=u2[:, :])
            cb_sb = cp.tile([128, AT, BL], f32)
            nc.sync.dma_start(out=cb_sb, in_=cb2[:, :, :])
            ident = cp.tile([128, 128], f32)
            make_identity(nc, ident)
            ones_sb = cp.tile([B + 1, 1], f32r)
            nc.vector.memset(ones_sb.bitcast(f32), 1.0)

            # ---- per-chunk softmax stats (combined once at the end)
            mall = cp.tile([1, BL, NCH], f32)
            lall = cp.tile([1, BL, NCH], f32)
            saccs = [cp.tile([128, KT, BL], f32, name=f"sacc{i}")
                     for i in range(NCH)]

            sel_v = sel.rearrange("j b (n s) -> j b n s", s=C)

            carries = []

            def phase1(i):
                hi_sb = hip.tile([128, KT, BL, C], f32r, tag="hi")
                nc.sync.dma_start(
                    out=hi_sb.rearrange("p t b s -> p (t b s)"), in_=hi5[i])
                sel_c = sp.tile([B + 1, BL, C], f32, tag="selc")
                nc.scalar.dma_start(out=sel_c, in_=sel_v[:, :, i, :])
                m_r = mp.tile([128, AT, BL, C], f32r, tag="m")
                for at in range(AT):
                    z_ps = pz.tile([128, BL, C], f32, tag="z")
                    for r in range(BL // 2):
                        for kt in range(KT):
                            nc.tensor.matmul(
                                z_ps[:, 2 * r : 2 * r + 2, :],
                                w1_sb[:, kt, at * 128 : (at + 1) * 128],
                                hi_sb[:, kt, 2 * r : 2 * r + 2, :],
                                start=(kt == 0), stop=(kt == KT - 1),
                            )
                    for b in range(BL):
                        nc.scalar.activation(
                            out=m_r[:, at, b, :], in_=z_ps[:, b, :],
                            func=mybir.ActivationFunctionType.Tanh,
                            bias=cb_sb[:, at, b : b + 1], scale=1.0,
                        )
                y_ps = py.tile([1, BL, C], f32, tag="y")
                for r in range(BL // 2):
                    for at in range(AT):
                        nc.tensor.matmul(
                            y_ps[:, 2 * r : 2 * r + 2, :],
                            u_sb[:, at : at + 1],
                            m_r[:, at, 2 * r : 2 * r + 2, :],
                            start=(at == 0), stop=(at == AT - 1),
                        )
                y_sb = sp.tile([1, BL, C], f32, tag="ysb", bufs=2)
                nc.scalar.activation(out=y_sb, in_=y_ps,
                                     func=mybir.ActivationFunctionType.Copy)

                ag_in = dp.tile([BL * C], f32, tag="agin")
                nc.scalar.dma_start(
                    out=ag_in.rearrange("(o n) -> o n", o=1),
                    in_=y_sb.rearrange("p b s -> p (b s)"))
                ag_out = dp.tile([B * C], f32, tag="agout",
                                 addr_space="Shared")
                nc.gpsimd.collective_compute(
                    "AllGather", mybir.AluOpType.bypass,
                    ins=[ag_in[:]], outs=[ag_out[:]],
                    replica_groups=[list(range(NCORES))],
                )
                y32 = sp.tile([B + 1, C], f32, tag="y32")
                nc.gpsimd.memset(y32[B : B + 1, :], 1.0)
                nc.scalar.dma_start(
                    out=y32[:B, :], in_=ag_out.rearrange("(j s) -> j s", s=C))
                return dict(hi_sb=hi_sb, sel_c=sel_c, y32=y32, i=i)

            def phase23(c):
                i = c["i"]
                sel_c, y32 = c["sel_c"], c["y32"]
                # masked_scatter selection: one-hot rows (plus a -1e20 mask
                # row) dotted with [y; 1]
                bt_ps = py.tile([1, BL, C], f32, tag="y")
                for b in range(BL):
                    selY = sp.tile([B + 1, C], f32r, tag="selY")
                    nc.vector.tensor_mul(selY, sel_c[:, b, :], y32)
                    nc.tensor.matmul(bt_ps[:, b, :], ones_sb, selY,
                                     start=True, stop=True)

                # chunk-local max -> no cross-chunk recurrence
                cmax = sp.tile([1, BL], f32, tag="cmax")
                nc.vector.tensor_reduce(
                    out=cmax.rearrange("p (b o) -> p b o", o=1), in_=bt_ps,
                    axis=mybir.AxisListType.X, op=mybir.AluOpType.max)
                nc.vector.tensor_copy(mall[:, :, i], cmax)
                nmnew = sp.tile([1, BL], f32, tag="nmnew")
                nc.vector.tensor_scalar_mul(nmnew, cmax, -1.0)
                e4 = sp.tile([1, BL, C], f32, tag="e4", bufs=2)
                for b in range(BL):
                    nc.scalar.activation(
                        out=e4[:, b, :], in_=bt_ps[:, b, :],
                        func=mybir.ActivationFunctionType.Exp,
                        bias=nmnew[:, b : b + 1], scale=1.0,
                        accum_out=lall[:, b, i : i + 1])

                sacc_i = saccs[i]
                ttr_scr = sp.tile([128, C], f32, tag="ttrscr")
                hi_sb = c["hi_sb"]
                for b in range(BL):
                    e_bc = ebp.tile([128, C], f32, tag=f"ebc{b}")
                    nc.gpsimd.partition_broadcast(e_bc, e4[:, b, :])
                    for kt in range(KT):
                        nc.vector._custom_dve(
                            TENSOR_TENSOR_REDUCE,
                            out=ttr_scr,
                            in0=hi_sb[:, kt, b, :].bitcast(f32),
                            in1=e_bc,
                            s0=0.0, s1=1.0,
                            accum_out=sacc_i[:, kt, b : b + 1],
                        )

            DEPTH = 2
            for i in range(NCH):
                carries.append(phase1(i))
                if len(carries) > DEPTH:
                    phase23(carries.pop(0))
            while carries:
                phase23(carries.pop(0))

            # ---- finalize: combine chunk partials, divide, transpose, store
            M = sp.tile([1, BL], f32, tag="cmax")
            nc.vector.tensor_reduce(
                out=M.rearrange("p (b o) -> p b o", o=1), in_=mall,
                axis=mybir.AxisListType.X, op=mybir.AluOpType.max)
            nM = sp.tile([1, BL], f32, tag="nmnew")
            nc.vector.tensor_scalar_mul(nM, M, -1.0)
            w = sp.tile([1, BL, NCH], f32, tag="w")
            for b in range(BL):
                nc.scalar.activation(
                    out=w[:, b, :], in_=mall[:, b, :],
                    func=mybir.ActivationFunctionType.Exp,
                    bias=nM[:, b : b + 1], scale=1.0)
            wl = sp.tile([1, BL, NCH], f32, tag="wl")
            nc.vector.tensor_mul(wl, w, lall)
            lsum = sp.tile([1, BL], f32, tag="lsum")
            nc.vector.tensor_reduce(
                out=lsum.rearrange("p (b o) -> p b o", o=1), in_=wl,
                axis=mybir.AxisListType.X, op=mybir.AluOpType.add)
            il = sp.tile([1, BL], f32, tag="il")
            nc.vector.reciprocal(il, lsum)
            wn = sp.tile([1, BL, NCH], f32, tag="wn")
            for b in range(BL):
                nc.vector.tensor_scalar_mul(wn[:, b, :], w[:, b, :],
                                            il[:, b : b + 1])
            sfin = sap.tile([128, KT, BL], f32, tag="sacc")
            for i in range(NCH):
                for b in range(BL):
                    wbc = ebp.tile([128, 1], f32, tag=f"cbc{b}")
                    nc.gpsimd.partition_broadcast(wbc, wn[:, b, i : i + 1])
                    if i == 0:
                        nc.vector.tensor_scalar_mul(
                            sfin[:, :, b], saccs[0][:, :, b], wbc[:, 0:1])
                    else:
                        tmp = sp.tile([128, KT], f32, tag="ftmp")
                        nc.vector.tensor_scalar_mul(
                            tmp, saccs[i][:, :, b], wbc[:, 0:1])
                        nc.vector.tensor_add(
                            sfin[:, :, b], sfin[:, :, b], tmp)
            t_ps = py.tile([KT * BL, 128], f32, tag="y")
            nc.tensor.transpose(
                t_ps, sfin.rearrange("p t b -> p (t b)"), ident)
            t_sb = sp.tile([KT * BL, 128], f32, tag="tsb")
            nc.vector.tensor_copy(t_sb, t_ps)
            for t in range(KT):
                nc.sync.dma_start(
                    out=out[:, t * 128 : (t + 1) * 128],
                    in_=t_sb[t * BL : (t + 1) * BL, :])

    nc.compile()
    _split_pe_waits(nc)
    return nc


def _split_pe_waits(nc):
    """TRN2 PE instructions (S3_LW encoding) take a single sync-wait slot.
    Bacc's legalization misses some Matmults; hoist excess waits onto
    dedicated PE NoOps inserted directly before the offender."""
    for f in nc.m.functions:
        for bb in f.blocks:
            insts = bb.instructions
            i = 0
            while i < len(insts):
                ins = insts[i]
                if type(ins).__name__ in ("InstMatmult", "InstNoOp") and \
                        ins.engine == mybir.EngineType.PE:
                    si = ins.sync_info
                    if si is not None and len(si.on_wait) > 1:
                        extra, keep = si.on_wait[:-1], si.on_wait[-1:]
                        for w in extra:
                            nop = mybir.InstNoOp(
                                name=nc.get_next_instruction_name(),
                                ins=[], outs=[])
                            nop.engine = ins.engine
                            nop.sync_info = mybir.SyncInfo(
                                on_wait=[w], on_update=[])
                            nc.register_instruction(nop)
                            insts.insert(i, nop)
                            i += 1
                        si.on_wait = keep
                i += 1


def prep_inputs(h_i, h_t, mask, W, b, u, S=S, H=H, A=A, C=256):
    """Shard + lay out the full inputs for the 8 cores."""
    h_i = np.asarray(h_i, np.float32)
    h_t = np.asarray(h_t, np.float32)
    mask = np.asarray(mask, bool)
    W = np.asarray(W, np.float32)
    b = np.asarray(b, np.float32)
    u = np.asarray(u, np.float32)

    KT = H // 128
    AT = A // 128
    NCH = S // C
    w1t = np.ascontiguousarray(W[:, :H].T)                      # [H, A]
    cb = h_t @ W[:, H:].T + b                                   # [B, A]
    cb2s = np.ascontiguousarray(
        cb.reshape(B, AT, 128).transpose(2, 1, 0))              # [128, AT, B]
    u2 = np.ascontiguousarray(u[:, 0].reshape(AT, 128).T)       # [128, AT]

    pos = np.clip(np.cumsum(mask.astype(np.int64), axis=0) - 1, 0, None)
    onehot = (np.arange(B)[None, :, None] == pos[:, None, :]) & mask[:, None, :]
    selall = onehot.astype(np.float32)                          # [B, B, S]
    negall = np.where(mask, np.float32(0), NEG).astype(np.float32)  # [B, S]
    sel33 = np.concatenate([selall, negall[:, None, :]], axis=1)  # [B, B+1, S]

    in_maps = []
    for c in range(NCORES):
        bs = slice(c * BL, (c + 1) * BL)
        # hi5[i, p, (t, b, s)] = h_i[b, i*C+s, t*128+p]
        hc = h_i[bs].reshape(BL, NCH, C, KT, 128)
        hi5 = np.ascontiguousarray(
            hc.transpose(1, 4, 3, 0, 2).reshape(NCH, 128, KT * BL * C))
        in_maps.append({
            "hi5": hi5,
            "w1t": w1t,
            "cb2": np.ascontiguousarray(cb2s[:, :, bs]),
            "u2": u2,
            "sel": np.ascontiguousarray(sel33[bs].transpose(1, 0, 2)),
        })
    return in_maps


_NC_CACHE = {}


def _get_nc():
    if "nc" not in _NC_CACHE:
        _NC_CACHE["nc"] = build_kernel()
    return _NC_CACHE["nc"]


def kernel(h_i, h_t, mask, W, b, u):
    nc = _get_nc()
    in_maps = prep_inputs(h_i, h_t, mask, W, b, u)
    res = run_bass_kernel_spmd(nc, in_maps, list(range(NCORES)))
    return np.concatenate([res.results[c]["out"] for c in range(NCORES)],
                          axis=0)


# revision 25
# speedup vs baseline: 1.1379x; 1.1379x over previous
"""Trainium2 Bass kernel for nn_Attention_b (tanh-attention with masked_scatter).

Data-parallel over batch: each of 8 NeuronCores owns 4 batches. Per core:
  phase 1  z = W1 @ h_i + (W2 @ h_t + b)   (fp32r GEMM, [A, rows])
           m = tanh(z); y = u . m          (raw scores, [rows])
  comm     AllGather of the per-chunk score slice across the 8 cores
  phase 2  masked_scatter selection (0/1 matrix against gathered scores)
           + online softmax over the sequence (flash-style, chunked)
  phase 3  s_acc += e * h_i  (fused DVE multiply-reduce on the resident
           h_i chunk -- h_i is read from HBM exactly once)
"""
import sys

for _p in ("/opt/trn_rl_repo",):
    if _p not in sys.path:
        sys.path.insert(0, _p)

import numpy as np

import concourse.bacc as bacc
import concourse.tile as tile
from concourse import mybir
from concourse.bass_utils import run_bass_kernel_spmd
from concourse.dve_ops import TENSOR_TENSOR_REDUCE
from concourse.masks import make_identity

NCORES = 8
B, S, H, A = 32, 2048, 1024, 256
BL = B // NCORES          # local batches per core
NEG = np.float32(-1e20)

f32 = mybir.dt.float32
f32r = mybir.dt.float32r


def build_kernel(S=S, H=H, A=A, C=256, hi_bufs=4):
    KT = H // 128             # contraction tiles
    AT = A // 128             # score tiles
    NCH = S // C              # sequence chunks
    assert S % C == 0 and H % 128 == 0 and A % 128 == 0 and C % 2 == 0

    nc = bacc.Bacc("TRN2", target_bir_lowering=False, debug=False,
                   num_devices=NCORES)

    # big operands are declared float32r (same bits as f32) so the plain
    # HWDGE DMA path can be used -- no SWDGE cast, no Q7 descriptor work
    hi5 = nc.declare_dram_parameter("hi5", [NCH, 128, KT * BL * C], f32r,
                                    isOutput=False)
    w1t = nc.declare_dram_parameter("w1t", [H, A], f32r, isOutput=False)
    cb2 = nc.declare_dram_parameter("cb2", [128, AT, BL], f32, isOutput=False)
    u2 = nc.declare_dram_parameter("u2", [128, AT], f32r, isOutput=False)
    sel = nc.declare_dram_parameter("sel", [B + 1, BL, S], f32,
                                    isOutput=False)
    out = nc.declare_dram_parameter("out", [BL, H], f32, isOutput=True)

    with tile.TileContext(nc) as tc:
        with (
            tc.tile_pool(name="consts", bufs=1) as cp,
            tc.tile_pool(name="hi", bufs=hi_bufs) as hip,
            tc.tile_pool(name="m", bufs=2) as mp,
            tc.tile_pool(name="small", bufs=3) as sp,
            tc.tile_pool(name="ebc", bufs=2) as ebp,
            tc.tile_pool(name="sacc", bufs=2) as sap,
            tc.tile_pool(name="pz", bufs=2, space="PSUM") as pz,
            tc.tile_pool(name="py", bufs=2, space="PSUM") as py,
            tc.tile_pool(name="dram", bufs=NCH, space="DRAM") as dp,
        ):
            # ---- preload replicated constants
            w1_sb = cp.tile([128, KT, A], f32r)
            nc.sync.dma_start(
                out=w1_sb, in_=w1t.rearrange("(t p) a -> p t a", p=128))
            u_sb = cp.tile([128, AT], f32r)
            nc.sync.dma_start(out=u_sb, in_=u2[:, :])
            cb_sb = cp.tile([128, AT, BL], f32)
            nc.sync.dma_start(out=cb_sb, in_=cb2[:, :, :])
            ident = cp.tile([128, 128], f32)
            make_identity(nc, ident)
            ones_sb = cp.tile([B + 1, 1], f32r)
            nc.vector.memset(ones_sb.bitcast(f32), 1.0)

            # ---- per-chunk softmax stats (combined once at the end)
            mall = cp.tile([1, BL, NCH], f32)
            lall = cp.tile([1, BL, NCH], f32)
            saccs = [cp.tile([128, KT, BL], f32, name=f"sacc{i}")
                     for i in range(NCH)]

            sel_v = sel.rearrange("j b (n s) -> j b n s", s=C)

            carries = []

            def phase1(i):
                hi_sb = hip.tile([128, KT, BL, C], f32r, tag="hi")
                nc.sync.dma_start(
                    out=hi_sb.rearrange("p t b s -> p (t b s)"), in_=hi5[i])
                sel_c = sp.tile([B + 1, BL, C], f32, tag="selc")
                nc.scalar.dma_start(out=sel_c, in_=sel_v[:, :, i, :])
                m_r = mp.tile([128, AT, BL, C], f32r, tag="m")
                for at in range(AT):
                    z_ps = pz.tile([128, BL, C], f32, tag="z")
                    for r in range(BL // 2):
                        for kt in range(KT):
                            nc.tensor.matmul(
                                z_ps[:, 2 * r : 2 * r + 2, :],
                                w1_sb[:, kt, at * 128 : (at + 1) * 128],
                                hi_sb[:, kt, 2 * r : 2 * r + 2, :],
                                start=(kt == 0), stop=(kt == KT - 1),
                            )
                    for b in range(BL):
                        nc.scalar.activation(
                            out=m_r[:, at, b, :], in_=z_ps[:, b, :],
                            func=mybir.ActivationFunctionType.Tanh,
                            bias=cb_sb[:, at, b : b + 1], scale=1.0,
                        )
                y_ps = py.tile([1, BL, C], f32, tag="y")
                for r in range(BL // 2):
                    for at in range(AT):
                        nc.tensor.matmul(
                            y_ps[:, 2 * r : 2 * r + 2, :],
                            u_sb[:, at : at + 1],
                            m_r[:, at, 2 * r : 2 * r + 2, :],
                            start=(at == 0), stop=(at == AT - 1),
                        )
                y_sb = sp.tile([1, BL, C], f32, tag="ysb", bufs=2)
                nc.scalar.activation(out=y_sb, in_=y_ps,
                                     func=mybir.ActivationFunctionType.Copy)

                ag_in = dp.tile([BL * C], f32, tag="agin")
                nc.scalar.dma_start(
                    out=ag_in.rearrange("(o n) -> o n", o=1),
                    in_=y_sb.rearrange("p b s -> p (b s)"))
                ag_out = dp.tile([B * C], f32, tag="agout",
                                 addr_space="Shared")
                nc.gpsimd.collective_compute(
                    "AllGather", mybir.AluOpType.bypass,
                    ins=[ag_in[:]], outs=[ag_out[:]],
                    replica_groups=[list(range(NCORES))],
                )
                y32 = sp.tile([B + 1, C], f32, tag="y32")
                nc.gpsimd.memset(y32[B : B + 1, :], 1.0)
                nc.scalar.dma_start(
                    out=y32[:B, :], in_=ag_out.rearrange("(j s) -> j s", s=C))
                return dict(hi_sb=hi_sb, sel_c=sel_c, y32=y32, i=i)

            def phase2(c):
                i = c["i"]
                sel_c, y32 = c["sel_c"], c["y32"]
                # masked_scatter selection: one-hot rows (plus a -1e20 mask
                # row) dotted with [y; 1]
                bt_ps = py.tile([1, BL, C], f32, tag="y")
                for b in range(BL):
                    selY = sp.tile([B + 1, C], f32r, tag="selY")
                    nc.vector.tensor_mul(selY, sel_c[:, b, :], y32)
                    nc.tensor.matmul(bt_ps[:, b, :], ones_sb, selY,
                                     start=True, stop=True)

                # chunk-local max -> no cross-chunk recurrence
                cmax = sp.tile([1, BL], f32, tag="cmax")
                nc.vector.tensor_reduce(
                    out=cmax.rearrange("p (b o) -> p b o", o=1), in_=bt_ps,
                    axis=mybir.AxisListType.X, op=mybir.AluOpType.max)
                nc.vector.tensor_copy(mall[:, :, i], cmax)
                nmnew = sp.tile([1, BL], f32, tag="nmnew")
                nc.vector.tensor_scalar_mul(nmnew, cmax, -1.0)
                e4 = sp.tile([1, BL, C], f32, tag="e4", bufs=2)
                ebcs = []
                for b in range(BL):
                    nc.scalar.activation(
                        out=e4[:, b, :], in_=bt_ps[:, b, :],
                        func=mybir.ActivationFunctionType.Exp,
                        bias=nmnew[:, b : b + 1], scale=1.0,
                        accum_out=lall[:, b, i : i + 1])
                    e_bc = ebp.tile([128, C], f32, tag=f"ebc{b}")
                    nc.gpsimd.partition_broadcast(e_bc, e4[:, b, :])
                    ebcs.append(e_bc)
                c["ebcs"] = ebcs

            def phase3(c):
                i = c["i"]
                sacc_i = saccs[i]
                ttr_scr = sp.tile([128, C], f32, tag="ttrscr")
                hi_sb = c["hi_sb"]
                for b in range(BL):
                    e_bc = c["ebcs"][b]
                    for kt in range(KT):
                        nc.vector._custom_dve(
                            TENSOR_TENSOR_REDUCE,
                            out=ttr_scr,
                            in0=hi_sb[:, kt, b, :].bitcast(f32),
                            in1=e_bc,
                            s0=0.0, s1=1.0,
                            accum_out=sacc_i[:, kt, b : b + 1],
                        )

            for i in range(NCH):
                carries.append(phase1(i))
                if len(carries) >= 2:
                    phase2(carries[-2])
                if len(carries) >= 3:
                    phase3(carries.pop(0))
            phase2(carries[-1])
            while carries:
                phase3(carries.pop(0))

            # ---- finalize: combine chunk partials, divide, transpose, store
            M = sp.tile([1, BL], f32, tag="cmax")
            nc.vector.tensor_reduce(
                out=M.rearrange("p (b o) -> p b o", o=1), in_=mall,
                axis=mybir.AxisListType.X, op=mybir.AluOpType.max)
            nM = sp.tile([1, BL], f32, tag="nmnew")
            nc.vector.tensor_scalar_mul(nM, M, -1.0)
            w = sp.tile([1, BL, NCH], f32, tag="w")
            for b in range(BL):
                nc.scalar.activation(
                    out=w[:, b, :], in_=mall[:, b, :],
                    func=mybir.ActivationFunctionType.Exp,
                    bias=nM[:, b : b + 1], scale=1.0)
            wl = sp.tile([1, BL, NCH], f32, tag="wl")
            nc.vector.tensor_mul(wl, w, lall)
            lsum = sp.tile([1, BL], f32, tag="lsum")
            nc.vector.tensor_reduce(
                out=lsum.rearrange("p (b o) -> p b o", o=1), in_=wl,
                axis=mybir.AxisListType.X, op=mybir.AluOpType.add)
            il = sp.tile([1, BL], f32, tag="il")
            nc.vector.reciprocal(il, lsum)
            wn = sp.tile([1, BL, NCH], f32, tag="wn")
            for b in range(BL):
                nc.vector.tensor_scalar_mul(wn[:, b, :], w[:, b, :],
                                            il[:, b : b + 1])
            sfin = sap.tile([128, KT, BL], f32, tag="sacc")
            for i in range(NCH):
                for b in range(BL):
                    wbc = ebp.tile([128, 1], f32, tag=f"cbc{b}")
                    nc.gpsimd.partition_broadcast(wbc, wn[:, b, i : i + 1])
                    if i == 0:
                        nc.vector.tensor_scalar_mul(
                            sfin[:, :, b], saccs[0][:, :, b], wbc[:, 0:1])
                    else:
                        tmp = sp.tile([128, KT], f32, tag="ftmp")
                        nc.vector.tensor_scalar_mul(
                            tmp, saccs[i][:, :, b], wbc[:, 0:1])
                        nc.vector.tensor_add(
                            sfin[:, :, b], sfin[:, :, b], tmp)
            t_ps = py.tile([KT * BL, 128], f32, tag="y")
            nc.tensor.transpose(
                t_ps, sfin.rearrange("p t b -> p (t b)"), ident)
            t_sb = sp.tile([KT * BL, 128], f32, tag="tsb")
            nc.vector.tensor_copy(t_sb, t_ps)
            for t in range(KT):
                nc.sync.dma_start(
                    out=out[:, t * 128 : (t + 1) * 128],
                    in_=t_sb[t * BL : (t + 1) * BL, :])

    nc.compile()
    _split_pe_waits(nc)
    return nc


def _split_pe_waits(nc):
    """TRN2 PE instructions (S3_LW encoding) take a single sync-wait slot.
    Bacc's legalization misses some Matmults; hoist excess waits onto
    dedicated PE NoOps inserted directly before the offender."""
    for f in nc.m.functions:
        for bb in f.blocks:
            insts = bb.instructions
            i = 0
            while i < len(insts):
                ins = insts[i]
                if type(ins).__name__ in ("InstMatmult", "InstNoOp") and \
                        ins.engine == mybir.EngineType.PE:
                    si = ins.sync_info
                    if si is not None and len(si.on_wait) > 1:
                        extra, keep = si.on_wait[:-1], si.on_wait[-1:]
                        for w in extra:
                            nop = mybir.InstNoOp(
                                name=nc.get_next_instruction_name(),
                                ins=[], outs=[])
                            nop.engine = ins.engine
                            nop.sync_info = mybir.SyncInfo(
                                on_wait=[w], on_update=[])
                            nc.register_instruction(nop)
                            insts.insert(i, nop)
                            i += 1
                        si.on_wait = keep
                i += 1


def prep_inputs(h_i, h_t, mask, W, b, u, S=S, H=H, A=A, C=256):
    """Shard + lay out the full inputs for the 8 cores."""
    h_i = np.asarray(h_i, np.float32)
    h_t = np.asarray(h_t, np.float32)
    mask = np.asarray(mask, bool)
    W = np.asarray(W, np.float32)
    b = np.asarray(b, np.float32)
    u = np.asarray(u, np.float32)

    KT = H // 128
    AT = A // 128
    NCH = S // C
    w1t = np.ascontiguousarray(W[:, :H].T)                      # [H, A]
    cb = h_t @ W[:, H:].T + b                                   # [B, A]
    cb2s = np.ascontiguousarray(
        cb.reshape(B, AT, 128).transpose(2, 1, 0))              # [128, AT, B]
    u2 = np.ascontiguousarray(u[:, 0].reshape(AT, 128).T)       # [128, AT]

    pos = np.clip(np.cumsum(mask.astype(np.int64), axis=0) - 1, 0, None)
    onehot = (np.arange(B)[None, :, None] == pos[:, None, :]) & mask[:, None, :]
    selall = onehot.astype(np.float32)                          # [B, B, S]
    negall = np.where(mask, np.float32(0), NEG).astype(np.float32)  # [B, S]
    sel33 = np.concatenate([selall, negall[:, None, :]], axis=1)  # [B, B+1, S]

    in_maps = []
    for c in range(NCORES):
        bs = slice(c * BL, (c + 1) * BL)
        # hi5[i, p, (t, b, s)] = h_i[b, i*C+s, t*128+p]
        hc = h_i[bs].reshape(BL, NCH, C, KT, 128)
        hi5 = np.ascontiguousarray(
            hc.transpose(1, 4, 3, 0, 2).reshape(NCH, 128, KT * BL * C))
        in_maps.append({
            "hi5": hi5,
            "w1t": w1t,
            "cb2": np.ascontiguousarray(cb2s[:, :, bs]),
            "u2": u2,
            "sel": np.ascontiguousarray(sel33[bs].transpose(1, 0, 2)),
        })
    return in_maps


_NC_CACHE = {}


def _get_nc():
    if "nc" not in _NC_CACHE:
        _NC_CACHE["nc"] = build_kernel()
    return _NC_CACHE["nc"]


def kernel(h_i, h_t, mask, W, b, u):
    nc = _get_nc()
    in_maps = prep_inputs(h_i, h_t, mask, W, b, u)
    res = run_bass_kernel_spmd(nc, in_maps, list(range(NCORES)))
    return np.concatenate([res.results[c]["out"] for c in range(NCORES)],
                          axis=0)
